# revision 1
# baseline (speedup 1.0000x reference)
"""Causal multi-head self-attention with RoPE on 8 TRN2 NeuronCores.

Sharding: data-parallel over batch (4) x tensor-parallel over heads (16 -> 2
groups of 8).  Core c handles batch c//2, head group c%2.  Each core computes
its 8 heads' attention and a partial O-projection (512 of the 1024 contraction
dims); the host sums the two partials per batch element.
"""

import os
import sys

import numpy as np

if "/opt/trn_rl_repo" not in sys.path:
    sys.path.insert(0, "/opt/trn_rl_repo")

D_MODEL = 1024
NUM_HEADS = 16
THETA = 10000.0
B, S = 4, 2048
DK = 64
HALF = DK // 2
P = 128
N_CORES = 8
HPC = 8                 # heads per core
DOUT = HPC * DK         # 512 per-core projected dims
KT = D_MODEL // P       # 8 contraction tiles
NSEQ = S // P           # 16 seq tiles of 128
NQB = S // 512          # 4 query blocks of 512
SCALE = 1.0 / np.sqrt(DK)

_CACHE = {}


def _build():
    """Build + compile the per-core Bass module (same program on all cores)."""
    import concourse.bass as bass
    import concourse.bacc as bacc
    import concourse.tile as tile
    import concourse.mybir as mybir
    from contextlib import ExitStack

    f32 = mybir.dt.float32
    bf16 = mybir.dt.bfloat16
    Exp = mybir.ActivationFunctionType.Exp

    nc = bacc.Bacc("TRN2", target_bir_lowering=False, debug=False,
                   enable_asserts=False, num_devices=N_CORES)

    xT = nc.dram_tensor("xT", [D_MODEL, S], bf16, kind="ExternalInput")
    wq = nc.dram_tensor("wq", [D_MODEL, DOUT], bf16, kind="ExternalInput")
    wk = nc.dram_tensor("wk", [D_MODEL, DOUT], bf16, kind="ExternalInput")
    wv = nc.dram_tensor("wv", [D_MODEL, DOUT], bf16, kind="ExternalInput")
    wo = nc.dram_tensor("wo", [DOUT, D_MODEL], bf16, kind="ExternalInput")
    cosn = nc.dram_tensor("cosn", [S, DK], f32, kind="ExternalInput")
    sinn = nc.dram_tensor("sinn", [S, DK], f32, kind="ExternalInput")
    maskt = nc.dram_tensor("maskt", [P, 4 * 512], bf16, kind="ExternalInput")
    ident = nc.dram_tensor("ident", [P, P], bf16, kind="ExternalInput")
    out = nc.dram_tensor("out", [S, D_MODEL], f32, kind="ExternalOutput")

    def rep8(ap):
        # replicate a [128, 64] tile 8x along free dim -> logical [128, 512]
        return bass.AP(tensor=ap.tensor, offset=ap.offset,
                       ap=[ap.ap[0], [0, HPC], [1, DK]])

    def pairswap(ap):
        # free-dim pair swap of a [128, 512] tile: (0,1,2,3,..)->(1,0,3,2,..)
        return bass.AP(tensor=ap.tensor, offset=ap.offset + 1,
                       ap=[ap.ap[0], [2, 256], [-1, 2]])

    with tile.TileContext(nc) as tc, ExitStack() as top:
        persist = top.enter_context(tc.tile_pool(name="persist", bufs=1))
        # psum pools (8 banks total): proj/oproj share 2, transpose 2,
        # scores 2, attention-accumulate 2
        mm_ps = top.enter_context(tc.tile_pool(name="mm_ps", bufs=2, space="PSUM"))
        tr_ps = top.enter_context(tc.tile_pool(name="tr_ps", bufs=2, space="PSUM"))
        sc_ps = top.enter_context(tc.tile_pool(name="sc_ps", bufs=2, space="PSUM"))
        av_ps = top.enter_context(tc.tile_pool(name="av_ps", bufs=2, space="PSUM"))
        ropet = top.enter_context(tc.tile_pool(name="ropet", bufs=2))
        natp = top.enter_context(tc.tile_pool(name="natp", bufs=4))
        ptp = top.enter_context(tc.tile_pool(name="ptp", bufs=4))
        rcpp = top.enter_context(tc.tile_pool(name="rcpp", bufs=2))
        rmatp = top.enter_context(tc.tile_pool(name="rmatp", bufs=2))
        ostg = top.enter_context(tc.tile_pool(name="ostg", bufs=3))

        # ---- persistent SBUF arrays ----
        # DMA emission order matters (single queue): interleave x/wq tiles so
        # the first Q-proj matmuls can start after ~1MB instead of ~5MB, then
        # rope tables, then the rest in first-use order.
        x_sb = [persist.tile([P, S], bf16, tag=f"x{k}", name=f"x{k}")
                for k in range(KT)]
        w_sb = {nm: [persist.tile([P, DOUT], bf16, tag=f"{nm}{k}",
                                  name=f"{nm}{k}") for k in range(KT)]
                for nm in ("wq", "wk", "wv")}
        wo_sb = [persist.tile([P, D_MODEL], bf16, tag=f"wo{k}", name=f"wo{k}")
                 for k in range(DOUT // P)]
        cos_sb = [persist.tile([P, DK], f32, tag=f"cos{m}", name=f"cos{m}")
                  for m in range(NSEQ)]
        sin_sb = [persist.tile([P, DK], f32, tag=f"sin{m}", name=f"sin{m}")
                  for m in range(NSEQ)]
        mask_sb = persist.tile([P, 4 * 512], bf16, tag="mask", name="mask")
        id_sb = persist.tile([P, P], bf16, tag="ident", name="ident")

        for k in range(KT):
            nc.sync.dma_start(out=x_sb[k], in_=xT[k * P:(k + 1) * P, :])
            nc.sync.dma_start(out=w_sb["wq"][k], in_=wq[k * P:(k + 1) * P, :])
        for m in range(8):
            nc.sync.dma_start(out=cos_sb[m], in_=cosn[m * P:(m + 1) * P, :])
            nc.sync.dma_start(out=sin_sb[m], in_=sinn[m * P:(m + 1) * P, :])
        nc.sync.dma_start(out=id_sb, in_=ident[:, :])
        for k in range(KT):
            nc.sync.dma_start(out=w_sb["wk"][k], in_=wk[k * P:(k + 1) * P, :])
        for m in range(8, NSEQ):
            nc.sync.dma_start(out=cos_sb[m], in_=cosn[m * P:(m + 1) * P, :])
            nc.sync.dma_start(out=sin_sb[m], in_=sinn[m * P:(m + 1) * P, :])
        for k in range(KT):
            nc.sync.dma_start(out=w_sb["wv"][k], in_=wv[k * P:(k + 1) * P, :])
        nc.sync.dma_start(out=mask_sb, in_=maskt[:, :])
        for k in range(DOUT // P):
            nc.sync.dma_start(out=wo_sb[k], in_=wo[k * P:(k + 1) * P, :])

        # outputs of phase A
        qt_sb = [persist.tile([P, S], bf16, tag=f"qt{d}", name=f"qt{d}")
                 for d in range(4)]
        kt_sb = [persist.tile([P, S], bf16, tag=f"kt{d}", name=f"kt{d}")
                 for d in range(4)]
        v_sb = [persist.tile([P, HPC * (DK + 1)], bf16, tag=f"v{t}", name=f"v{t}")
                for t in range(NSEQ)]
        ot_sb = [persist.tile([P, S], bf16, tag=f"ot{d}", name=f"ot{d}")
                 for d in range(4)]

        # ---- group-interleaved pipeline over seq groups g (4 m-tiles each) --

        def proj_group(g):
            """Q/K/V projections + rope + transpose for m in [4g, 4g+4)."""
            for nm, dst in (("wq", qt_sb), ("wk", kt_sb)):
                pend = None          # lag-1 transpose drain: (trt, m)
                for m in range(4 * g, 4 * g + 4):
                    ps = mm_ps.tile([P, DOUT], f32, tag="mm", name="mm")
                    for k in range(KT):
                        nc.tensor.matmul(ps, x_sb[k][:, m * P:(m + 1) * P],
                                         w_sb[nm][k], start=(k == 0),
                                         stop=(k == KT - 1))
                    t1 = ropet.tile([P, DOUT], f32, tag="rt1", name="rt1")
                    t2 = ropet.tile([P, DOUT], f32, tag="rt2", name="rt2")
                    nc.vector.tensor_mul(t1, ps, rep8(cos_sb[m]))
                    nc.vector.tensor_mul(t2, pairswap(ps), rep8(sin_sb[m]))
                    nat = natp.tile([P, DOUT], bf16, tag="nat", name="nat")
                    nc.vector.tensor_add(nat, t1, t2)
                    if pend is not None:
                        ptr, pm = pend
                        for d in range(4):
                            nc.vector.tensor_copy(
                                dst[d][:, pm * P:(pm + 1) * P],
                                ptr[:, d * P:(d + 1) * P])
                    trt = tr_ps.tile([P, 512], bf16, tag="tr", name="trt")
                    for d in range(4):
                        nc.tensor.transpose(trt[:, d * P:(d + 1) * P],
                                            nat[:, d * P:(d + 1) * P], id_sb)
                    pend = (trt, m)
                ptr, pm = pend
                for d in range(4):
                    nc.vector.tensor_copy(dst[d][:, pm * P:(pm + 1) * P],
                                          ptr[:, d * P:(d + 1) * P])
            for m in range(4 * g, 4 * g + 4):
                ps = mm_ps.tile([P, DOUT], f32, tag="mm", name="mm")
                for k in range(KT):
                    nc.tensor.matmul(ps, x_sb[k][:, m * P:(m + 1) * P],
                                     w_sb["wv"][k], start=(k == 0),
                                     stop=(k == KT - 1))
                vt = v_sb[m]
                ones_ap = bass.AP(tensor=vt.tensor, offset=vt.offset + DK,
                                  ap=[vt.ap[0], [DK + 1, HPC]])
                nc.gpsimd.memset(ones_ap, 1.0)
                vcols = bass.AP(tensor=vt.tensor, offset=vt.offset,
                                ap=[vt.ap[0], [DK + 1, HPC], [1, DK]])
                nc.scalar.copy(vcols, ps)

        def attn_group(g):
            """Attention for query block qb=g over all heads.

            Diagonal kv-tiles (t in [4g, 4g+4)) only have valid scores for
            q-cols >= 128*(t%4): scores+exp are trimmed to that span, and the
            full-width 0/1 mask-mul zeroes both the stale prefix and the
            intra-tile upper triangle of pt before the full-width AV matmul.
            """
            cols = slice(g * 512, (g + 1) * 512)
            for h in range(HPC):
                db, po = h // 2, (h % 2) * DK
                av = av_ps.tile([DK + 1, 512], f32, tag="av", name="av")
                nt = 4 * g + 4
                for t in range(nt):
                    v = t - 4 * g
                    c0 = 128 * v if v >= 0 else 0
                    sc = sc_ps.tile([P, 512], f32, tag="sc", name="sc")
                    nc.tensor.matmul(
                        sc[:, c0:], kt_sb[db][po:po + DK, t * P:(t + 1) * P],
                        qt_sb[db][po:po + DK, g * 512 + c0:(g + 1) * 512],
                        start=True, stop=True)
                    pt = ptp.tile([P, 512], bf16, tag="pt", name="pt")
                    nc.scalar.activation(pt[:, c0:], sc[:, c0:], Exp)
                    if v >= 0:
                        nc.vector.tensor_mul(
                            pt, pt, mask_sb[:, v * 512:(v + 1) * 512])
                    nc.tensor.matmul(
                        av, v_sb[t][:, h * (DK + 1):(h + 1) * (DK + 1)],
                        pt, start=(t == 0), stop=(t == nt - 1))
                rcp = rcpp.tile([1, 512], f32, tag="rcp", name="rcp")
                nc.vector.reciprocal(rcp, av[DK:DK + 1, :])
                rmat = rmatp.tile([DK, 512], f32, tag="rmat", name="rmat")
                nc.gpsimd.partition_broadcast(rmat, rcp, channels=DK)
                nc.vector.tensor_mul(ot_sb[db][po:po + DK, cols],
                                     av[0:DK, :], rmat)

        def oproj_group(g):
            for m in range(4 * g, 4 * g + 4):
                for nb in range(2):
                    ps = mm_ps.tile([P, 512], f32, tag="mm", name="mm")
                    for k in range(4):
                        nc.tensor.matmul(
                            ps, ot_sb[k][:, m * P:(m + 1) * P],
                            wo_sb[k][:, nb * 512:(nb + 1) * 512],
                            start=(k == 0), stop=(k == 3))
                    og = ostg.tile([P, 512], f32, tag="og", name="og")
                    nc.vector.tensor_copy(og, ps)
                    nc.sync.dma_start(
                        out=out[m * P:(m + 1) * P, nb * 512:(nb + 1) * 512],
                        in_=og)

        # zero the pt pool slots once: trimmed exp leaves stale prefixes that
        # the mask-mul reads (0 * garbage must not be 0 * NaN)
        for _ in range(4):
            ptz = ptp.tile([P, 512], bf16, tag="pt", name="ptz")
            nc.gpsimd.memset(ptz, 0.0)

        # software-staged emission: keep PE fed with proj work while the
        # ACT-heavy attention of earlier groups drains
        proj_group(0)
        proj_group(1)
        for g in range(4):
            attn_group(g)
            if g + 2 < 4:
                proj_group(g + 2)
            oproj_group(g)

    nc.compile()
    return nc


def _get_nc():
    if "nc" not in _CACHE:
        _CACHE["nc"] = _build()
    return _CACHE["nc"]


def _prep_core_inputs(q_proj_weight, k_proj_weight, v_proj_weight,
                      o_proj_weight, in_features, token_positions):
    """Host-side sharding: returns the list of 8 per-core input dicts."""
    import ml_dtypes
    bf = ml_dtypes.bfloat16

    x = np.asarray(in_features, np.float32)
    wqf = np.asarray(q_proj_weight, np.float32)
    wkf = np.asarray(k_proj_weight, np.float32)
    wvf = np.asarray(v_proj_weight, np.float32)
    wof = np.asarray(o_proj_weight, np.float32)
    tp = np.asarray(token_positions).astype(np.float64)

    inv = 1.0 / (THETA ** (np.arange(HALF, dtype=np.float64) / HALF))
    fr = tp[:, None] * inv[None, :]                       # [S, 32]
    cosn = np.repeat(np.cos(fr), 2, axis=1).astype(np.float32)  # [S, 64]
    sg = np.tile(np.array([-1.0, 1.0]), HALF)[None, :]
    sinn = (np.repeat(np.sin(fr), 2, axis=1) * sg).astype(np.float32)

    kv = np.arange(P)[:, None]
    qc = np.arange(512)[None, :]
    maskt = np.concatenate(
        [(qc >= 128 * v + kv) for v in range(4)], axis=1).astype(bf)

    identity = np.eye(P, dtype=bf)

    in_maps = []
    for c in range(N_CORES):
        b, hg = c // 2, c % 2
        rows = slice(hg * DOUT, (hg + 1) * DOUT)
        wv_s = wvf[rows].T.astype(bf)                      # [1024, 512]
        in_maps.append({
            "xT": np.ascontiguousarray(x[b].T).astype(bf),
            "wq": np.ascontiguousarray((wqf[rows] * SCALE).T).astype(bf),
            "wk": np.ascontiguousarray(wkf[rows].T).astype(bf),
            "wv": np.ascontiguousarray(wv_s),
            "wo": np.ascontiguousarray(wof[:, rows].T).astype(bf),
            "cosn": cosn,
            "sinn": sinn,
            "maskt": maskt,
            "ident": identity,
        })
    return in_maps


def kernel(q_proj_weight, k_proj_weight, v_proj_weight, o_proj_weight,
           in_features, token_positions):
    from concourse.bass_utils import run_bass_kernel_spmd

    nc = _get_nc()
    in_maps = _prep_core_inputs(q_proj_weight, k_proj_weight, v_proj_weight,
                                o_proj_weight, in_features, token_positions)
    trace = bool(int(os.environ.get("KBENCH_TRACE", "0")))
    res = run_bass_kernel_spmd(nc, in_maps, list(range(N_CORES)), trace=trace)
    _CACHE["last_results"] = res
    if res.exec_time_ns is not None:
        _CACHE["exec_time_ns"] = res.exec_time_ns

    outp = np.empty((B, S, D_MODEL), np.float32)
    for b in range(B):
        outp[b] = res.results[2 * b]["out"] + res.results[2 * b + 1]["out"]
    return outp



# revision 5
# speedup vs baseline: 1.3453x; 1.3453x over previous
"""Causal multi-head self-attention with RoPE on 8 TRN2 NeuronCores (v8).

Sharding: data-parallel over batch (4) x tensor-parallel over heads (16 -> 2
groups of 8).  Core c handles batch c//2, head group c%2.  Each core computes
its 8 heads' attention and a partial O-projection (512 of the 1024 contraction
dims); the host sums the two partials per batch element.

Structure:
- Q/K projected directly transposed (out = W_tile^T x^T): no PE transposes.
- Q/K weight rows de-interleaved per head ([e0..e31, o0..o31]) so RoPE pair
  mixing is a 32-row partition swap done on SBUF temps (DVE + Pool split).
- Score matmuls for a head pair emitted adjacently at base partitions 0/64
  -> tile_position (0,0)/(64,0), concurrent on the 128x128 PE array.
- Both heads' scores land in one 2-bank PSUM tile; ONE exp per kv-tile.
- Diagonal kv-tiles trimmed to cols >= c0 through scores/exp/AV; the 0/1
  mask-mul only touches the [128, 2, 128] diagonal sub-blocks.
- Persistent inputs live in merged SBUF tiles so the whole input load is
  ~11 large DMAs (per-DMA queue overhead, not bandwidth, is the limiter).
- Fine-grained software pipelining: proj/O-proj matmul quanta are emitted
  between attention kv-steps as PE filler.
- Output bf16; host sums the two partial O-projections per batch in fp32.
"""

import os
import sys

import numpy as np

if "/opt/trn_rl_repo" not in sys.path:
    sys.path.insert(0, "/opt/trn_rl_repo")

D_MODEL = 1024
NUM_HEADS = 16
THETA = 10000.0
B, S = 4, 2048
DK = 64
HALF = DK // 2
P = 128
N_CORES = 8
HPC = 8                 # heads per core
DOUT = HPC * DK         # 512 per-core projected dims
KT = D_MODEL // P       # 8 contraction tiles
NSEQ = S // P           # 16 seq tiles of 128
SCALE = 1.0 / np.sqrt(DK)

_CACHE = {}


def _build():
    import concourse.bass as bass
    import concourse.bacc as bacc
    import concourse.tile as tile
    import concourse.mybir as mybir
    from contextlib import ExitStack

    f32 = mybir.dt.float32
    bf16 = mybir.dt.bfloat16
    Exp = mybir.ActivationFunctionType.Exp

    nc = bacc.Bacc("TRN2", target_bir_lowering=False, debug=False,
                   enable_asserts=False, num_devices=N_CORES)

    xT = nc.dram_tensor("xT", [D_MODEL, S], bf16, kind="ExternalInput")
    wq = nc.dram_tensor("wq", [D_MODEL, DOUT], bf16, kind="ExternalInput")
    wk = nc.dram_tensor("wk", [D_MODEL, DOUT], bf16, kind="ExternalInput")
    wv = nc.dram_tensor("wv", [D_MODEL, DOUT], bf16, kind="ExternalInput")
    wo = nc.dram_tensor("wo", [DOUT, D_MODEL], bf16, kind="ExternalInput")
    ctab = nc.dram_tensor("ctab", [P, S], bf16, kind="ExternalInput")
    stab = nc.dram_tensor("stab", [P, S], bf16, kind="ExternalInput")
    maskt = nc.dram_tensor("maskt", [P, P], bf16, kind="ExternalInput")
    swapm = nc.dram_tensor("swapm", [P, P], bf16, kind="ExternalInput")
    out = nc.dram_tensor("out", [S, D_MODEL], bf16, kind="ExternalOutput")

    def dram3(t, k_count, row_block, c0, width, row_len):
        """DRAM view [p, k, j] = t[row_block*k + p, c0 + j], j < width."""
        return bass.AP(tensor=t, offset=c0,
                       ap=[[row_len, P], [row_block * row_len, k_count],
                           [1, width]])

    with tile.TileContext(nc) as tc, ExitStack() as top:
        persist = top.enter_context(tc.tile_pool(name="persist", bufs=1))
        # PSUM budget (8 banks): proj/oproj 2, scores 2x2, attn-accum 2
        mm_ps = top.enter_context(tc.tile_pool(name="mm_ps", bufs=2, space="PSUM"))
        sc_ps = top.enter_context(tc.tile_pool(name="sc_ps", bufs=2, space="PSUM"))
        av_ps = top.enter_context(tc.tile_pool(name="av_ps", bufs=1, space="PSUM"))
        ropet = top.enter_context(tc.tile_pool(name="ropet", bufs=2))
        ptp = top.enter_context(tc.tile_pool(name="ptp", bufs=3))
        normp = top.enter_context(tc.tile_pool(name="normp", bufs=2))
        ostg = top.enter_context(tc.tile_pool(name="ostg", bufs=2))

        # ---- persistent SBUF arrays (merged per tensor: 1 DMA each) ----
        x_all = persist.tile([P, KT * S], bf16, tag="x", name="x")
        wq_all = persist.tile([P, KT * DOUT], bf16, tag="wq", name="wq")
        wk_all = persist.tile([P, KT * DOUT], bf16, tag="wk", name="wk")
        wv_all = persist.tile([P, KT * DOUT], bf16, tag="wv", name="wv")
        wo_all = persist.tile([P, 4 * D_MODEL], bf16, tag="wo", name="wo")
        ctab_sb = persist.tile([P, S], bf16, tag="ctab", name="ctab")
        stab_sb = persist.tile([P, S], bf16, tag="stab", name="stab")
        mask_sb = persist.tile([P, P], bf16, tag="mask", name="mask")
        swap_sb = persist.tile([P, P], bf16, tag="swapm", name="swapm")

        def xs(k, a, b):
            return x_all[:, k * S + a:k * S + b]

        def ws(w_t, k, a, b):
            return w_t[:, k * DOUT + a:k * DOUT + b]

        def x_chunk_dma(c, k0=0, k1=KT):
            dst = bass.AP(tensor=x_all.tensor,
                          offset=x_all.offset + k0 * S + 512 * c,
                          ap=[x_all.ap[0], [S, k1 - k0], [1, 512]])
            src = bass.AP(tensor=xT, offset=k0 * P * S + 512 * c,
                          ap=[[S, P], [P * S, k1 - k0], [1, 512]])
            nc.sync.dma_start(out=dst, in_=src)

        def w_dma(dst, src, q, k0=0, k1=KT):
            # whole projection weight, 1KB per-partition lines per k-tile
            dd = bass.AP(tensor=dst.tensor, offset=dst.offset + k0 * DOUT,
                         ap=[dst.ap[0], [DOUT, k1 - k0], [1, DOUT]])
            sa = bass.AP(tensor=src, offset=k0 * P * DOUT,
                         ap=[[DOUT, P], [P * DOUT, k1 - k0], [1, DOUT]])
            q.dma_start(out=dd, in_=sa)

        # DMA order == consumption order (the sim's DMA engine is serial;
        # on HW the SP/ACT split still gives two queues).  The first x/wq
        # pieces are split so the opening matmul chain starts early.
        x_chunk_dma(0, 0, 2)
        w_dma(wq_all, wq, nc.scalar, 0, 2)
        x_chunk_dma(0, 2, KT)
        w_dma(wq_all, wq, nc.scalar, 2, KT)
        w_dma(wk_all, wk, nc.scalar)
        nc.scalar.dma_start(out=ctab_sb, in_=ctab[:, :])
        nc.scalar.dma_start(out=stab_sb, in_=stab[:, :])
        x_chunk_dma(1)
        nc.scalar.dma_start(out=swap_sb, in_=swapm[:, :])
        w_dma(wv_all, wv, nc.sync)
        nc.scalar.dma_start(out=mask_sb, in_=maskt[:, :])
        x_chunk_dma(2)
        x_chunk_dma(3)
        wo_d = bass.AP(tensor=wo_all.tensor, offset=wo_all.offset,
                       ap=[wo_all.ap[0], [D_MODEL, 4], [1, D_MODEL]])
        nc.scalar.dma_start(out=wo_d,
                            in_=dram3(wo, 4, P, 0, D_MODEL, D_MODEL))

        # phase-A outputs: transposed roped q/k [dims, S] (2 heads per tile,
        # rows de-interleaved per 64-row head block), v in [S-tile,
        # 8*(dk+1)] with a ones column per head, ot [dims, S].
        qt_sb = [persist.tile([P, S], bf16, tag=f"qt{d}", name=f"qt{d}")
                 for d in range(4)]
        kt_sb = [persist.tile([P, S], bf16, tag=f"kt{d}", name=f"kt{d}")
                 for d in range(4)]
        v_sb = [persist.tile([P, HPC * (DK + 1)], bf16, tag=f"v{t}", name=f"v{t}")
                for t in range(NSEQ)]
        ot_sb = [persist.tile([P, S], bf16, tag=f"ot{d}", name=f"ot{d}")
                 for d in range(4)]

        for m in range(NSEQ):
            vt = v_sb[m]
            ones_ap = bass.AP(tensor=vt.tensor, offset=vt.offset + DK,
                              ap=[vt.ap[0], [DK + 1, HPC]])
            nc.gpsimd.memset(ones_ap, 1.0)

        # rope pipelining state: the 32-row pair swap is a permutation-matrix
        # matmul (engines can't cross partitions; PE can).  The swap-matmul +
        # final add for quantum i are emitted at quantum i+1 so PE never
        # waits on quantum i's DVE muls.
        pend = {"s": None}

        def flush_pend():
            if pend["s"] is None:
                return
            t1, u, dstap = pend["s"]
            pend["s"] = None
            us = mm_ps.tile([P, 512], f32, tag="mm", name="us")
            nc.tensor.matmul(us, swap_sb, u, start=True, stop=True)
            nc.vector.tensor_add(dstap, t1, us)

        def qk_quantum(g, d, w_t, dst):
            """One d-tile of a transposed Q or K projection + its rope."""
            a, b = g * 512, (g + 1) * 512
            ps = mm_ps.tile([P, 512], f32, tag="mm", name="mm")
            for k in range(KT):
                nc.tensor.matmul(ps, ws(w_t, k, d * P, (d + 1) * P),
                                 xs(k, a, b), start=(k == 0),
                                 stop=(k == KT - 1))
            flush_pend()
            # rope: out = ps*C + Pswap @ (ps*S2)
            t1 = ropet.tile([P, 512], f32, tag="t1", name="t1")
            u = ropet.tile([P, 512], bf16, tag="u", name="u")
            nc.vector.tensor_mul(t1, ps, ctab_sb[:, a:b])
            nc.vector.tensor_mul(u, ps, stab_sb[:, a:b])
            pend["s"] = (t1, u, dst[d][:, a:b])

        def v_quantum(m):
            """V projection for one S-tile m."""
            flush_pend()
            ps = mm_ps.tile([P, 512], f32, tag="mm", name="mm")
            for k in range(KT):
                nc.tensor.matmul(ps, xs(k, m * P, (m + 1) * P),
                                 ws(wv_all, k, 0, DOUT), start=(k == 0),
                                 stop=(k == KT - 1))
            vt = v_sb[m]
            vcols = bass.AP(tensor=vt.tensor, offset=vt.offset,
                            ap=[vt.ap[0], [DK + 1, HPC], [1, DK]])
            nc.scalar.copy(vcols, ps)

        def proj_quanta(g):
            for d in range(4):
                for w_t, dst in ((wq_all, qt_sb), (wk_all, kt_sb)):
                    yield lambda g=g, d=d, w_t=w_t, dst=dst: \
                        qk_quantum(g, d, w_t, dst)
            for m in range(4 * g, 4 * g + 4):
                yield lambda m=m: v_quantum(m)

        def proj_group(g):
            for q in proj_quanta(g):
                q()

        def attn_group(g, only_pair=None, step=lambda: None):
            """Attention for query block g (512 cols) over head pairs."""
            gq = g * 512
            nt = 4 * g + 4
            flush_pend()
            pairs = range(4) if only_pair is None else [only_pair]
            for p in pairs:
                av = av_ps.tile([DK + 1, 1024], f32, tag="av", name="av")
                for t in range(nt):
                    step()
                    v = t - 4 * g
                    c0 = 128 * v if v > 0 else 0
                    w = 512 - c0
                    sc = sc_ps.tile([P, 1024], f32, tag="sc", name="sc")
                    for hh in range(2):
                        nc.tensor.matmul(
                            sc[:, 512 * hh + c0:512 * hh + 512],
                            kt_sb[p][64 * hh:64 * hh + 64,
                                     t * P:(t + 1) * P],
                            qt_sb[p][64 * hh:64 * hh + 64,
                                     gq + c0:gq + 512],
                            start=True, stop=True)
                    pt = ptp.tile([P, 1024], bf16, tag="pt", name="pt")
                    sc_v = bass.AP(tensor=sc.tensor, offset=sc.offset + c0,
                                   ap=[sc.ap[0], [512, 2], [1, w]])
                    pt_v = bass.AP(tensor=pt.tensor, offset=pt.offset + c0,
                                   ap=[pt.ap[0], [512, 2], [1, w]])
                    nc.scalar.activation(pt_v, sc_v, Exp)
                    if v >= 0:
                        pm = bass.AP(tensor=pt.tensor, offset=pt.offset + c0,
                                     ap=[pt.ap[0], [512, 2], [1, P]])
                        mk = bass.AP(tensor=mask_sb.tensor,
                                     offset=mask_sb.offset,
                                     ap=[mask_sb.ap[0], [0, 2], [1, P]])
                        nc.vector.tensor_mul(pm, pm, mk)
                    for hh in range(2):
                        nc.tensor.matmul(
                            av[:, 512 * hh + c0:512 * hh + 512],
                            v_sb[t][:, (2 * p + hh) * (DK + 1):
                                    (2 * p + hh) * (DK + 1) + DK + 1],
                            pt[:, 512 * hh + c0:512 * hh + 512],
                            start=(t == 0), stop=(t == nt - 1))
                # evacuate av to SBUF promptly so the next pair's first AV
                # matmul doesn't wait on the whole norm chain reading PSUM
                avs = normp.tile([DK + 1, 1024], f32, tag="avs", name="avs")
                nc.vector.tensor_copy(avs[:, 0:512], av[:, 0:512])
                nc.vector.tensor_copy(avs[:, 512:1024], av[:, 512:1024])
                rcp = normp.tile([1, 1024], f32, tag="rcp", name="rcp")
                nc.vector.reciprocal(rcp, avs[DK:DK + 1, :])
                rmat = normp.tile([DK, 1024], f32, tag="rmat", name="rmat")
                nc.gpsimd.partition_broadcast(rmat, rcp, channels=DK)
                nc.vector.tensor_mul(ot_sb[p][0:64, gq:gq + 512],
                                     avs[0:DK, 0:512], rmat[:, 0:512])
                # engines can't write other partitions; DMA moves the odd
                # head's half down to rows 64-127
                nrm2 = normp.tile([DK, 512], bf16, tag="nrm2", name="nrm2")
                nc.vector.tensor_mul(nrm2, avs[0:DK, 512:1024],
                                     rmat[:, 512:1024])
                q = nc.sync if p % 2 == 0 else nc.scalar
                q.dma_start(out=ot_sb[p][64:128, gq:gq + 512], in_=nrm2)

        og_live = {}

        def oproj_quantum(m, nb):
            flush_pend()
            ps = mm_ps.tile([P, 512], f32, tag="mm", name="mm")
            for k in range(4):
                nc.tensor.matmul(
                    ps, ot_sb[k][:, m * P:(m + 1) * P],
                    wo_all[:, k * D_MODEL + nb * 512:
                           k * D_MODEL + (nb + 1) * 512],
                    start=(k == 0), stop=(k == 3))
            if nb == 0:
                og_live[m] = ostg.tile([P, D_MODEL], bf16, tag="og",
                                       name="og")
            og = og_live[m]
            nc.vector.tensor_copy(og[:, nb * 512:(nb + 1) * 512], ps)
            if nb == 1:
                q = nc.sync if m % 2 == 0 else nc.scalar
                q.dma_start(out=out[m * P:(m + 1) * P, :],
                            in_=og_live.pop(m))

        def oproj_quanta(g):
            for m in range(4 * g, 4 * g + 4):
                for nb in range(2):
                    yield lambda m=m, nb=nb: oproj_quantum(m, nb)

        # Fine-grained software pipelining: engines run their streams in
        # strict emission order, so proj(2,3)/O-proj matmuls are emitted as
        # filler quanta INTO the attn kv-loops.
        for _rep in range(int(os.environ.get("BODY_REPEAT", "1"))):
            if _rep:
                for c in range(4):
                    x_chunk_dma(c)

            fill = []
            state = {"n": 0, "cad": 3}

            def step():
                state["n"] += 1
                if state["n"] % state["cad"] == 0 and fill:
                    fill.pop(0)()

            proj_group(0)
            proj_group(1)
            fill.extend(proj_quanta(2))
            attn_group(0, step=step)
            fill.extend(proj_quanta(3))
            attn_group(1, step=step)
            fill.extend(oproj_quanta(0))
            attn_group(2, step=step)
            fill.extend(oproj_quanta(1))
            state["cad"] = 2
            for p in range(4):
                attn_group(3, only_pair=p, step=step)
                if p >= 2:
                    fill.extend(oproj_quanta(p))
            while fill:
                fill.pop(0)()

    nc.compile()
    return nc


def _get_nc():
    if "nc" not in _CACHE:
        _CACHE["nc"] = _build()
    return _CACHE["nc"]


def _prep_core_inputs(q_proj_weight, k_proj_weight, v_proj_weight,
                      o_proj_weight, in_features, token_positions):
    """Host-side sharding: returns the list of 8 per-core input dicts."""
    import ml_dtypes
    bf = ml_dtypes.bfloat16

    x = np.asarray(in_features, np.float32)
    wqf = np.asarray(q_proj_weight, np.float32)
    wkf = np.asarray(k_proj_weight, np.float32)
    wvf = np.asarray(v_proj_weight, np.float32)
    wof = np.asarray(o_proj_weight, np.float32)
    tp = np.asarray(token_positions).astype(np.float64)

    # de-interleave permutation within each head: [e0..e31, o0..o31]
    perm = np.concatenate(
        [64 * h + np.concatenate([2 * np.arange(32), 2 * np.arange(32) + 1])
         for h in range(HPC)])

    inv = 1.0 / (THETA ** (np.arange(HALF, dtype=np.float64) / HALF))
    fr = tp[:, None] * inv[None, :]                      # [S, 32]
    cosT = np.cos(fr).T                                  # [32, S]
    sinT = np.sin(fr).T
    ctab = np.tile(cosT, (4, 1)).astype(bf)              # [128, S]
    stab = np.concatenate([sinT, -sinT, sinT, -sinT], axis=0).astype(bf)

    kv = np.arange(P)[:, None]
    j = np.arange(P)[None, :]
    maskt = (j >= kv).astype(bf)                         # [128, 128]
    swapm = (kv == (j ^ 32)).astype(bf)                  # 32-row pair swap

    in_maps = []
    for c in range(N_CORES):
        b, hg = c // 2, c % 2
        rows = slice(hg * DOUT, (hg + 1) * DOUT)
        in_maps.append({
            "xT": np.ascontiguousarray(x[b].T).astype(bf),
            "wq": np.ascontiguousarray((wqf[rows][perm] * SCALE).T).astype(bf),
            "wk": np.ascontiguousarray(wkf[rows][perm].T).astype(bf),
            "wv": np.ascontiguousarray(wvf[rows].T).astype(bf),
            "wo": np.ascontiguousarray(wof[:, rows].T).astype(bf),
            "ctab": ctab,
            "stab": stab,
            "maskt": maskt,
            "swapm": swapm,
        })
    return in_maps


def kernel(q_proj_weight, k_proj_weight, v_proj_weight, o_proj_weight,
           in_features, token_positions):
    from concourse.bass_utils import run_bass_kernel_spmd

    nc = _get_nc()
    in_maps = _prep_core_inputs(q_proj_weight, k_proj_weight, v_proj_weight,
                                o_proj_weight, in_features, token_positions)
    trace = bool(int(os.environ.get("KBENCH_TRACE", "0")))
    res = run_bass_kernel_spmd(nc, in_maps, list(range(N_CORES)), trace=trace)
    _CACHE["last_results"] = res
    if res.exec_time_ns is not None:
        _CACHE["exec_time_ns"] = res.exec_time_ns

    outp = np.empty((B, S, D_MODEL), np.float32)
    for b in range(B):
        outp[b] = (res.results[2 * b]["out"].astype(np.float32)
                   + res.results[2 * b + 1]["out"].astype(np.float32))
    return outp


# revision 6
# speedup vs baseline: 1.3580x; 1.0095x over previous
"""Causal multi-head self-attention with RoPE on 8 TRN2 NeuronCores (v8).

Sharding: data-parallel over batch (4) x tensor-parallel over heads (16 -> 2
groups of 8).  Core c handles batch c//2, head group c%2.  Each core computes
its 8 heads' attention and a partial O-projection (512 of the 1024 contraction
dims); the host sums the two partials per batch element.

Structure:
- Q/K projected directly transposed (out = W_tile^T x^T): no PE transposes.
- Q/K weight rows de-interleaved per head ([e0..e31, o0..o31]) so RoPE pair
  mixing is a 32-row partition swap done on SBUF temps (DVE + Pool split).
- Score matmuls for a head pair emitted adjacently at base partitions 0/64
  -> tile_position (0,0)/(64,0), concurrent on the 128x128 PE array.
- Both heads' scores land in one 2-bank PSUM tile; ONE exp per kv-tile.
- Diagonal kv-tiles trimmed to cols >= c0 through scores/exp/AV; the 0/1
  mask-mul only touches the [128, 2, 128] diagonal sub-blocks.
- Persistent inputs live in merged SBUF tiles so the whole input load is
  ~11 large DMAs (per-DMA queue overhead, not bandwidth, is the limiter).
- Fine-grained software pipelining: proj/O-proj matmul quanta are emitted
  between attention kv-steps as PE filler.
- Output bf16; host sums the two partial O-projections per batch in fp32.
"""

import os
import sys

import numpy as np

if "/opt/trn_rl_repo" not in sys.path:
    sys.path.insert(0, "/opt/trn_rl_repo")

D_MODEL = 1024
NUM_HEADS = 16
THETA = 10000.0
B, S = 4, 2048
DK = 64
HALF = DK // 2
P = 128
N_CORES = 8
HPC = 8                 # heads per core
DOUT = HPC * DK         # 512 per-core projected dims
KT = D_MODEL // P       # 8 contraction tiles
NSEQ = S // P           # 16 seq tiles of 128
SCALE = 1.0 / np.sqrt(DK)

_CACHE = {}


def _build():
    import concourse.bass as bass
    import concourse.bacc as bacc
    import concourse.tile as tile
    import concourse.mybir as mybir
    from contextlib import ExitStack

    f32 = mybir.dt.float32
    bf16 = mybir.dt.bfloat16
    Exp = mybir.ActivationFunctionType.Exp

    nc = bacc.Bacc("TRN2", target_bir_lowering=False, debug=False,
                   enable_asserts=False, num_devices=N_CORES)

    xT = nc.dram_tensor("xT", [D_MODEL, S], bf16, kind="ExternalInput")
    wq = nc.dram_tensor("wq", [D_MODEL, DOUT], bf16, kind="ExternalInput")
    wk = nc.dram_tensor("wk", [D_MODEL, DOUT], bf16, kind="ExternalInput")
    wv = nc.dram_tensor("wv", [D_MODEL, DOUT], bf16, kind="ExternalInput")
    wo = nc.dram_tensor("wo", [DOUT, D_MODEL], bf16, kind="ExternalInput")
    ctab = nc.dram_tensor("ctab", [P, S], bf16, kind="ExternalInput")
    stab = nc.dram_tensor("stab", [P, S], bf16, kind="ExternalInput")
    maskt = nc.dram_tensor("maskt", [P, P], bf16, kind="ExternalInput")
    swapm = nc.dram_tensor("swapm", [P, P], bf16, kind="ExternalInput")
    out = nc.dram_tensor("out", [S, D_MODEL], bf16, kind="ExternalOutput")

    def dram3(t, k_count, row_block, c0, width, row_len):
        """DRAM view [p, k, j] = t[row_block*k + p, c0 + j], j < width."""
        return bass.AP(tensor=t, offset=c0,
                       ap=[[row_len, P], [row_block * row_len, k_count],
                           [1, width]])

    with tile.TileContext(nc) as tc, ExitStack() as top:
        persist = top.enter_context(tc.tile_pool(name="persist", bufs=1))
        # PSUM budget (8 banks): proj/oproj 2, scores 2x2, attn-accum 2
        mm_ps = top.enter_context(tc.tile_pool(name="mm_ps", bufs=2, space="PSUM"))
        sc_ps = top.enter_context(tc.tile_pool(name="sc_ps", bufs=2, space="PSUM"))
        av_ps = top.enter_context(tc.tile_pool(name="av_ps", bufs=1, space="PSUM"))
        ropet = top.enter_context(tc.tile_pool(name="ropet", bufs=2))
        ptp = top.enter_context(tc.tile_pool(name="ptp", bufs=3))
        normp = top.enter_context(tc.tile_pool(name="normp", bufs=2))
        ostg = top.enter_context(tc.tile_pool(name="ostg", bufs=2))

        # ---- persistent SBUF arrays (merged per tensor: 1 DMA each) ----
        x_all = persist.tile([P, KT * S], bf16, tag="x", name="x")
        wq_all = persist.tile([P, KT * DOUT], bf16, tag="wq", name="wq")
        wk_all = persist.tile([P, KT * DOUT], bf16, tag="wk", name="wk")
        wv_all = persist.tile([P, KT * DOUT], bf16, tag="wv", name="wv")
        wo_all = persist.tile([P, 4 * D_MODEL], bf16, tag="wo", name="wo")
        ctab_sb = persist.tile([P, S], bf16, tag="ctab", name="ctab")
        stab_sb = persist.tile([P, S], bf16, tag="stab", name="stab")
        mask_sb = persist.tile([P, P], bf16, tag="mask", name="mask")
        swap_sb = persist.tile([P, P], bf16, tag="swapm", name="swapm")

        def xs(k, a, b):
            return x_all[:, k * S + a:k * S + b]

        def ws(w_t, k, a, b):
            return w_t[:, k * DOUT + a:k * DOUT + b]

        def x_chunk_dma(c, k0=0, k1=KT):
            dst = bass.AP(tensor=x_all.tensor,
                          offset=x_all.offset + k0 * S + 512 * c,
                          ap=[x_all.ap[0], [S, k1 - k0], [1, 512]])
            src = bass.AP(tensor=xT, offset=k0 * P * S + 512 * c,
                          ap=[[S, P], [P * S, k1 - k0], [1, 512]])
            nc.sync.dma_start(out=dst, in_=src)

        def w_dma(dst, src, q, k0=0, k1=KT):
            # whole projection weight, 1KB per-partition lines per k-tile
            dd = bass.AP(tensor=dst.tensor, offset=dst.offset + k0 * DOUT,
                         ap=[dst.ap[0], [DOUT, k1 - k0], [1, DOUT]])
            sa = bass.AP(tensor=src, offset=k0 * P * DOUT,
                         ap=[[DOUT, P], [P * DOUT, k1 - k0], [1, DOUT]])
            q.dma_start(out=dd, in_=sa)

        # DMA order == consumption order (the sim's DMA engine is serial;
        # on HW the SP/ACT split still gives two queues).  The first x/wq
        # pieces are split so the opening matmul chain starts early.
        x_chunk_dma(0, 0, 2)
        w_dma(wq_all, wq, nc.scalar, 0, 2)
        nc.scalar.dma_start(out=swap_sb, in_=swapm[:, :])
        x_chunk_dma(0, 2, KT)
        w_dma(wq_all, wq, nc.scalar, 2, KT)
        w_dma(wk_all, wk, nc.scalar)
        nc.scalar.dma_start(out=ctab_sb, in_=ctab[:, :])
        nc.scalar.dma_start(out=stab_sb, in_=stab[:, :])
        nc.scalar.dma_start(out=mask_sb, in_=maskt[:, :])
        w_dma(wv_all, wv, nc.sync)
        x_chunk_dma(1)
        x_chunk_dma(2)
        x_chunk_dma(3)
        wo_d = bass.AP(tensor=wo_all.tensor, offset=wo_all.offset,
                       ap=[wo_all.ap[0], [D_MODEL, 4], [1, D_MODEL]])
        nc.scalar.dma_start(out=wo_d,
                            in_=dram3(wo, 4, P, 0, D_MODEL, D_MODEL))

        # phase-A outputs: transposed roped q/k [dims, S] (2 heads per tile,
        # rows de-interleaved per 64-row head block), v in [S-tile,
        # 8*(dk+1)] with a ones column per head, ot [dims, S].
        qt_sb = [persist.tile([P, S], bf16, tag=f"qt{d}", name=f"qt{d}")
                 for d in range(4)]
        kt_sb = [persist.tile([P, S], bf16, tag=f"kt{d}", name=f"kt{d}")
                 for d in range(4)]
        v_sb = [persist.tile([P, HPC * (DK + 1)], bf16, tag=f"v{t}", name=f"v{t}")
                for t in range(NSEQ)]
        ot_sb = [persist.tile([P, S], bf16, tag=f"ot{d}", name=f"ot{d}")
                 for d in range(4)]

        for m in range(NSEQ):
            vt = v_sb[m]
            ones_ap = bass.AP(tensor=vt.tensor, offset=vt.offset + DK,
                              ap=[vt.ap[0], [DK + 1, HPC]])
            nc.gpsimd.memset(ones_ap, 1.0)

        # rope pipelining state: the 32-row pair swap is a permutation-matrix
        # matmul (engines can't cross partitions; PE can).  The swap-matmul +
        # final add for quantum i are emitted at quantum i+1 so PE never
        # waits on quantum i's DVE muls.
        pend = {"s": None}

        def flush_pend():
            if pend["s"] is None:
                return
            t1, u, dstap = pend["s"]
            pend["s"] = None
            us = mm_ps.tile([P, 512], f32, tag="mm", name="us")
            nc.tensor.matmul(us, swap_sb, u, start=True, stop=True)
            nc.vector.tensor_add(dstap, t1, us)

        def qk_quantum(g, d, w_t, dst):
            """One d-tile of a transposed Q or K projection + its rope."""
            a, b = g * 512, (g + 1) * 512
            ps = mm_ps.tile([P, 512], f32, tag="mm", name="mm")
            for k in range(KT):
                nc.tensor.matmul(ps, ws(w_t, k, d * P, (d + 1) * P),
                                 xs(k, a, b), start=(k == 0),
                                 stop=(k == KT - 1))
            flush_pend()
            # rope: out = ps*C + Pswap @ (ps*S2)
            t1 = ropet.tile([P, 512], f32, tag="t1", name="t1")
            u = ropet.tile([P, 512], bf16, tag="u", name="u")
            nc.vector.tensor_mul(t1, ps, ctab_sb[:, a:b])
            nc.vector.tensor_mul(u, ps, stab_sb[:, a:b])
            pend["s"] = (t1, u, dst[d][:, a:b])

        def v_quantum(m):
            """V projection for one S-tile m."""
            flush_pend()
            ps = mm_ps.tile([P, 512], f32, tag="mm", name="mm")
            for k in range(KT):
                nc.tensor.matmul(ps, xs(k, m * P, (m + 1) * P),
                                 ws(wv_all, k, 0, DOUT), start=(k == 0),
                                 stop=(k == KT - 1))
            vt = v_sb[m]
            vcols = bass.AP(tensor=vt.tensor, offset=vt.offset,
                            ap=[vt.ap[0], [DK + 1, HPC], [1, DK]])
            nc.scalar.copy(vcols, ps)

        def proj_quanta(g):
            for d in range(4):
                for w_t, dst in ((wq_all, qt_sb), (wk_all, kt_sb)):
                    yield lambda g=g, d=d, w_t=w_t, dst=dst: \
                        qk_quantum(g, d, w_t, dst)
            for m in range(4 * g, 4 * g + 4):
                yield lambda m=m: v_quantum(m)

        def proj_group(g):
            for q in proj_quanta(g):
                q()

        def attn_group(g, only_pair=None, step=lambda: None):
            """Attention for query block g (512 cols) over head pairs."""
            gq = g * 512
            nt = 4 * g + 4
            flush_pend()
            pairs = range(4) if only_pair is None else [only_pair]
            for p in pairs:
                av = av_ps.tile([DK + 1, 1024], f32, tag="av", name="av")
                for t in range(nt):
                    step()
                    v = t - 4 * g
                    c0 = 128 * v if v > 0 else 0
                    w = 512 - c0
                    sc = sc_ps.tile([P, 1024], f32, tag="sc", name="sc")
                    for hh in range(2):
                        nc.tensor.matmul(
                            sc[:, 512 * hh + c0:512 * hh + 512],
                            kt_sb[p][64 * hh:64 * hh + 64,
                                     t * P:(t + 1) * P],
                            qt_sb[p][64 * hh:64 * hh + 64,
                                     gq + c0:gq + 512],
                            start=True, stop=True)
                    pt = ptp.tile([P, 1024], bf16, tag="pt", name="pt")
                    sc_v = bass.AP(tensor=sc.tensor, offset=sc.offset + c0,
                                   ap=[sc.ap[0], [512, 2], [1, w]])
                    pt_v = bass.AP(tensor=pt.tensor, offset=pt.offset + c0,
                                   ap=[pt.ap[0], [512, 2], [1, w]])
                    nc.scalar.activation(pt_v, sc_v, Exp)
                    if v >= 0:
                        pm = bass.AP(tensor=pt.tensor, offset=pt.offset + c0,
                                     ap=[pt.ap[0], [512, 2], [1, P]])
                        mk = bass.AP(tensor=mask_sb.tensor,
                                     offset=mask_sb.offset,
                                     ap=[mask_sb.ap[0], [0, 2], [1, P]])
                        nc.vector.tensor_mul(pm, pm, mk)
                    for hh in range(2):
                        nc.tensor.matmul(
                            av[:, 512 * hh + c0:512 * hh + 512],
                            v_sb[t][:, (2 * p + hh) * (DK + 1):
                                    (2 * p + hh) * (DK + 1) + DK + 1],
                            pt[:, 512 * hh + c0:512 * hh + 512],
                            start=(t == 0), stop=(t == nt - 1))
                # evacuate av to SBUF promptly so the next pair's first AV
                # matmul doesn't wait on the whole norm chain reading PSUM
                avs = normp.tile([DK + 1, 1024], f32, tag="avs", name="avs")
                nc.vector.tensor_copy(avs[:, 0:512], av[:, 0:512])
                nc.vector.tensor_copy(avs[:, 512:1024], av[:, 512:1024])
                rcp = normp.tile([1, 1024], f32, tag="rcp", name="rcp")
                nc.vector.reciprocal(rcp, avs[DK:DK + 1, :])
                rmat = normp.tile([DK, 1024], f32, tag="rmat", name="rmat")
                nc.gpsimd.partition_broadcast(rmat, rcp, channels=DK)
                nc.vector.tensor_mul(ot_sb[p][0:64, gq:gq + 512],
                                     avs[0:DK, 0:512], rmat[:, 0:512])
                # engines can't write other partitions; DMA moves the odd
                # head's half down to rows 64-127
                nrm2 = normp.tile([DK, 512], bf16, tag="nrm2", name="nrm2")
                nc.vector.tensor_mul(nrm2, avs[0:DK, 512:1024],
                                     rmat[:, 512:1024])
                q = nc.sync if p % 2 == 0 else nc.scalar
                q.dma_start(out=ot_sb[p][64:128, gq:gq + 512], in_=nrm2)

        og_live = {}

        def oproj_quantum(m, nb):
            flush_pend()
            ps = mm_ps.tile([P, 512], f32, tag="mm", name="mm")
            for k in range(4):
                nc.tensor.matmul(
                    ps, ot_sb[k][:, m * P:(m + 1) * P],
                    wo_all[:, k * D_MODEL + nb * 512:
                           k * D_MODEL + (nb + 1) * 512],
                    start=(k == 0), stop=(k == 3))
            if nb == 0:
                og_live[m] = ostg.tile([P, D_MODEL], bf16, tag="og",
                                       name="og")
            og = og_live[m]
            nc.vector.tensor_copy(og[:, nb * 512:(nb + 1) * 512], ps)
            if nb == 1:
                q = nc.sync if m % 2 == 0 else nc.scalar
                q.dma_start(out=out[m * P:(m + 1) * P, :],
                            in_=og_live.pop(m))

        def oproj_quanta(g):
            for m in range(4 * g, 4 * g + 4):
                for nb in range(2):
                    yield lambda m=m, nb=nb: oproj_quantum(m, nb)

        # Fine-grained software pipelining: engines run their streams in
        # strict emission order, so proj(2,3)/O-proj matmuls are emitted as
        # filler quanta INTO the attn kv-loops.
        for _rep in range(int(os.environ.get("BODY_REPEAT", "1"))):
            if _rep:
                for c in range(4):
                    x_chunk_dma(c)

            fill = []
            state = {"n": 0, "cad": 3}

            def step():
                state["n"] += 1
                if state["n"] % state["cad"] == 0 and fill:
                    fill.pop(0)()

            proj_group(0)
            proj_group(1)
            fill.extend(proj_quanta(2))
            attn_group(0, step=step)
            fill.extend(proj_quanta(3))
            attn_group(1, step=step)
            fill.extend(oproj_quanta(0))
            attn_group(2, step=step)
            fill.extend(oproj_quanta(1))
            state["cad"] = 2
            for p in range(4):
                attn_group(3, only_pair=p, step=step)
                if p >= 2:
                    fill.extend(oproj_quanta(p))
            while fill:
                fill.pop(0)()

    nc.compile()
    return nc


def _get_nc():
    if "nc" not in _CACHE:
        _CACHE["nc"] = _build()
    return _CACHE["nc"]


def _prep_core_inputs(q_proj_weight, k_proj_weight, v_proj_weight,
                      o_proj_weight, in_features, token_positions):
    """Host-side sharding: returns the list of 8 per-core input dicts."""
    import ml_dtypes
    bf = ml_dtypes.bfloat16

    x = np.asarray(in_features, np.float32)
    wqf = np.asarray(q_proj_weight, np.float32)
    wkf = np.asarray(k_proj_weight, np.float32)
    wvf = np.asarray(v_proj_weight, np.float32)
    wof = np.asarray(o_proj_weight, np.float32)
    tp = np.asarray(token_positions).astype(np.float64)

    # de-interleave permutation within each head: [e0..e31, o0..o31]
    perm = np.concatenate(
        [64 * h + np.concatenate([2 * np.arange(32), 2 * np.arange(32) + 1])
         for h in range(HPC)])

    inv = 1.0 / (THETA ** (np.arange(HALF, dtype=np.float64) / HALF))
    fr = tp[:, None] * inv[None, :]                      # [S, 32]
    cosT = np.cos(fr).T                                  # [32, S]
    sinT = np.sin(fr).T
    ctab = np.tile(cosT, (4, 1)).astype(bf)              # [128, S]
    stab = np.concatenate([sinT, -sinT, sinT, -sinT], axis=0).astype(bf)

    kv = np.arange(P)[:, None]
    j = np.arange(P)[None, :]
    maskt = (j >= kv).astype(bf)                         # [128, 128]
    swapm = (kv == (j ^ 32)).astype(bf)                  # 32-row pair swap

    in_maps = []
    for c in range(N_CORES):
        b, hg = c // 2, c % 2
        rows = slice(hg * DOUT, (hg + 1) * DOUT)
        in_maps.append({
            "xT": np.ascontiguousarray(x[b].T).astype(bf),
            "wq": np.ascontiguousarray((wqf[rows][perm] * SCALE).T).astype(bf),
            "wk": np.ascontiguousarray(wkf[rows][perm].T).astype(bf),
            "wv": np.ascontiguousarray(wvf[rows].T).astype(bf),
            "wo": np.ascontiguousarray(wof[:, rows].T).astype(bf),
            "ctab": ctab,
            "stab": stab,
            "maskt": maskt,
            "swapm": swapm,
        })
    return in_maps


def kernel(q_proj_weight, k_proj_weight, v_proj_weight, o_proj_weight,
           in_features, token_positions):
    from concourse.bass_utils import run_bass_kernel_spmd

    nc = _get_nc()
    in_maps = _prep_core_inputs(q_proj_weight, k_proj_weight, v_proj_weight,
                                o_proj_weight, in_features, token_positions)
    trace = bool(int(os.environ.get("KBENCH_TRACE", "0")))
    res = run_bass_kernel_spmd(nc, in_maps, list(range(N_CORES)), trace=trace)
    _CACHE["last_results"] = res
    if res.exec_time_ns is not None:
        _CACHE["exec_time_ns"] = res.exec_time_ns

    outp = np.empty((B, S, D_MODEL), np.float32)
    for b in range(B):
        outp[b] = (res.results[2 * b]["out"].astype(np.float32)
                   + res.results[2 * b + 1]["out"].astype(np.float32))
    return outp


# revision 7
# speedup vs baseline: 1.3943x; 1.0267x over previous
"""Causal multi-head self-attention with RoPE on 8 TRN2 NeuronCores (v8).

Sharding: data-parallel over batch (4) x tensor-parallel over heads (16 -> 2
groups of 8).  Core c handles batch c//2, head group c%2.  Each core computes
its 8 heads' attention and a partial O-projection (512 of the 1024 contraction
dims); the host sums the two partials per batch element.

Structure:
- Q/K projected directly transposed (out = W_tile^T x^T): no PE transposes.
- Q/K weight rows de-interleaved per head ([e0..e31, o0..o31]) so RoPE pair
  mixing is a 32-row partition swap done on SBUF temps (DVE + Pool split).
- Score matmuls for a head pair emitted adjacently at base partitions 0/64
  -> tile_position (0,0)/(64,0), concurrent on the 128x128 PE array.
- Both heads' scores land in one 2-bank PSUM tile; ONE exp per kv-tile.
- Diagonal kv-tiles trimmed to cols >= c0 through scores/exp/AV; the 0/1
  mask-mul only touches the [128, 2, 128] diagonal sub-blocks.
- Persistent inputs live in merged SBUF tiles so the whole input load is
  ~11 large DMAs (per-DMA queue overhead, not bandwidth, is the limiter).
- Fine-grained software pipelining: proj/O-proj matmul quanta are emitted
  between attention kv-steps as PE filler.
- Output bf16; host sums the two partial O-projections per batch in fp32.
"""

import os
import sys

import numpy as np

if "/opt/trn_rl_repo" not in sys.path:
    sys.path.insert(0, "/opt/trn_rl_repo")

D_MODEL = 1024
NUM_HEADS = 16
THETA = 10000.0
B, S = 4, 2048
DK = 64
HALF = DK // 2
P = 128
N_CORES = 8
HPC = 8                 # heads per core
DOUT = HPC * DK         # 512 per-core projected dims
KT = D_MODEL // P       # 8 contraction tiles
NSEQ = S // P           # 16 seq tiles of 128
SCALE = 1.0 / np.sqrt(DK)

_CACHE = {}


def _build():
    import concourse.bass as bass
    import concourse.bacc as bacc
    import concourse.tile as tile
    import concourse.mybir as mybir
    from contextlib import ExitStack

    f32 = mybir.dt.float32
    bf16 = mybir.dt.bfloat16
    Exp = mybir.ActivationFunctionType.Exp

    nc = bacc.Bacc("TRN2", target_bir_lowering=False, debug=False,
                   enable_asserts=False, num_devices=N_CORES)

    xT = nc.dram_tensor("xT", [D_MODEL, S], bf16, kind="ExternalInput")
    wq = nc.dram_tensor("wq", [D_MODEL, DOUT], bf16, kind="ExternalInput")
    wk = nc.dram_tensor("wk", [D_MODEL, DOUT], bf16, kind="ExternalInput")
    wv = nc.dram_tensor("wv", [D_MODEL, DOUT], bf16, kind="ExternalInput")
    wo = nc.dram_tensor("wo", [DOUT, D_MODEL], bf16, kind="ExternalInput")
    ctab = nc.dram_tensor("ctab", [P, S], bf16, kind="ExternalInput")
    stab = nc.dram_tensor("stab", [P, S], bf16, kind="ExternalInput")
    maskt = nc.dram_tensor("maskt", [P, P], bf16, kind="ExternalInput")
    swapm = nc.dram_tensor("swapm", [P, P], bf16, kind="ExternalInput")
    out = nc.dram_tensor("out", [S, D_MODEL], bf16, kind="ExternalOutput")

    def dram3(t, k_count, row_block, c0, width, row_len):
        """DRAM view [p, k, j] = t[row_block*k + p, c0 + j], j < width."""
        return bass.AP(tensor=t, offset=c0,
                       ap=[[row_len, P], [row_block * row_len, k_count],
                           [1, width]])

    with tile.TileContext(nc) as tc, ExitStack() as top:
        persist = top.enter_context(tc.tile_pool(name="persist", bufs=1))
        # PSUM budget (8 banks): proj/oproj 2, scores 2x2, attn-accum 2
        mm_ps = top.enter_context(tc.tile_pool(name="mm_ps", bufs=2, space="PSUM"))
        sc_ps = top.enter_context(tc.tile_pool(name="sc_ps", bufs=2, space="PSUM"))
        av_ps = top.enter_context(tc.tile_pool(name="av_ps", bufs=1, space="PSUM"))
        ropet = top.enter_context(tc.tile_pool(name="ropet", bufs=2))
        ptp = top.enter_context(tc.tile_pool(name="ptp", bufs=3))
        normp = top.enter_context(tc.tile_pool(name="normp", bufs=2))
        ostg = top.enter_context(tc.tile_pool(name="ostg", bufs=2))

        # ---- persistent SBUF arrays (merged per tensor: 1 DMA each) ----
        x_all = persist.tile([P, KT * S], bf16, tag="x", name="x")
        wq_all = persist.tile([P, KT * DOUT], bf16, tag="wq", name="wq")
        wk_all = persist.tile([P, KT * DOUT], bf16, tag="wk", name="wk")
        wv_all = persist.tile([P, KT * DOUT], bf16, tag="wv", name="wv")
        wo_all = persist.tile([P, 4 * D_MODEL], bf16, tag="wo", name="wo")
        ctab_sb = persist.tile([P, S], bf16, tag="ctab", name="ctab")
        stab_sb = persist.tile([P, S], bf16, tag="stab", name="stab")
        mask_sb = persist.tile([P, P], bf16, tag="mask", name="mask")
        swap_sb = persist.tile([P, P], bf16, tag="swapm", name="swapm")

        def xs(k, a, b):
            return x_all[:, k * S + a:k * S + b]

        def ws(w_t, k, a, b):
            return w_t[:, k * DOUT + a:k * DOUT + b]

        def x_chunk_dma(c, k0=0, k1=KT):
            dst = bass.AP(tensor=x_all.tensor,
                          offset=x_all.offset + k0 * S + 512 * c,
                          ap=[x_all.ap[0], [S, k1 - k0], [1, 512]])
            src = bass.AP(tensor=xT, offset=k0 * P * S + 512 * c,
                          ap=[[S, P], [P * S, k1 - k0], [1, 512]])
            nc.sync.dma_start(out=dst, in_=src)

        def w_dma(dst, src, q, k0=0, k1=KT):
            # whole projection weight, 1KB per-partition lines per k-tile
            dd = bass.AP(tensor=dst.tensor, offset=dst.offset + k0 * DOUT,
                         ap=[dst.ap[0], [DOUT, k1 - k0], [1, DOUT]])
            sa = bass.AP(tensor=src, offset=k0 * P * DOUT,
                         ap=[[DOUT, P], [P * DOUT, k1 - k0], [1, DOUT]])
            q.dma_start(out=dd, in_=sa)

        # DMA order == consumption order (the sim's DMA engine is serial;
        # on HW the SP/ACT split still gives two queues).  The first x/wq
        # pieces are split so the opening matmul chain starts early.
        x_chunk_dma(0)
        w_dma(wq_all, wq, nc.scalar)
        nc.scalar.dma_start(out=swap_sb, in_=swapm[:, :])
        w_dma(wk_all, wk, nc.scalar)
        nc.scalar.dma_start(out=ctab_sb, in_=ctab[:, :])
        nc.scalar.dma_start(out=stab_sb, in_=stab[:, :])
        nc.scalar.dma_start(out=mask_sb, in_=maskt[:, :])
        w_dma(wv_all, wv, nc.sync)
        x_chunk_dma(1)
        x_chunk_dma(2)
        x_chunk_dma(3)
        wo_d = bass.AP(tensor=wo_all.tensor, offset=wo_all.offset,
                       ap=[wo_all.ap[0], [D_MODEL, 4], [1, D_MODEL]])
        nc.scalar.dma_start(out=wo_d,
                            in_=dram3(wo, 4, P, 0, D_MODEL, D_MODEL))

        # phase-A outputs: transposed roped q/k [dims, S] (2 heads per tile,
        # rows de-interleaved per 64-row head block), v in [S-tile,
        # 8*(dk+1)] with a ones column per head, ot [dims, S].
        qt_sb = [persist.tile([P, S], bf16, tag=f"qt{d}", name=f"qt{d}")
                 for d in range(4)]
        kt_sb = [persist.tile([P, S], bf16, tag=f"kt{d}", name=f"kt{d}")
                 for d in range(4)]
        v_sb = [persist.tile([P, HPC * (DK + 1)], bf16, tag=f"v{t}", name=f"v{t}")
                for t in range(NSEQ)]
        ot_sb = [persist.tile([P, S], bf16, tag=f"ot{d}", name=f"ot{d}")
                 for d in range(4)]

        for m in range(NSEQ):
            vt = v_sb[m]
            ones_ap = bass.AP(tensor=vt.tensor, offset=vt.offset + DK,
                              ap=[vt.ap[0], [DK + 1, HPC]])
            nc.gpsimd.memset(ones_ap, 1.0)

        # rope pipelining state: the 32-row pair swap is a permutation-matrix
        # matmul (engines can't cross partitions; PE can).  The swap-matmul +
        # final add for quantum i are emitted at quantum i+1 so PE never
        # waits on quantum i's DVE muls.
        pend = {"s": None}

        def flush_pend():
            if pend["s"] is None:
                return
            t1, u, dstap = pend["s"]
            pend["s"] = None
            us = mm_ps.tile([P, 512], f32, tag="mm", name="us")
            nc.tensor.matmul(us, swap_sb, u, start=True, stop=True)
            nc.vector.tensor_add(dstap, t1, us)

        def qk_quantum(g, d, w_t, dst):
            """One d-tile of a transposed Q or K projection + its rope."""
            a, b = g * 512, (g + 1) * 512
            ps = mm_ps.tile([P, 512], f32, tag="mm", name="mm")
            for k in range(KT):
                nc.tensor.matmul(ps, ws(w_t, k, d * P, (d + 1) * P),
                                 xs(k, a, b), start=(k == 0),
                                 stop=(k == KT - 1))
            flush_pend()
            # rope: out = ps*C + Pswap @ (ps*S2)
            t1 = ropet.tile([P, 512], f32, tag="t1", name="t1")
            u = ropet.tile([P, 512], bf16, tag="u", name="u")
            nc.vector.tensor_mul(t1, ps, ctab_sb[:, a:b])
            nc.vector.tensor_mul(u, ps, stab_sb[:, a:b])
            pend["s"] = (t1, u, dst[d][:, a:b])

        def v_quantum(m):
            """V projection for one S-tile m."""
            flush_pend()
            ps = mm_ps.tile([P, 512], f32, tag="mm", name="mm")
            for k in range(KT):
                nc.tensor.matmul(ps, xs(k, m * P, (m + 1) * P),
                                 ws(wv_all, k, 0, DOUT), start=(k == 0),
                                 stop=(k == KT - 1))
            vt = v_sb[m]
            vcols = bass.AP(tensor=vt.tensor, offset=vt.offset,
                            ap=[vt.ap[0], [DK + 1, HPC], [1, DK]])
            nc.scalar.copy(vcols, ps)

        def proj_quanta(g):
            for d in range(4):
                for w_t, dst in ((wq_all, qt_sb), (wk_all, kt_sb)):
                    yield lambda g=g, d=d, w_t=w_t, dst=dst: \
                        qk_quantum(g, d, w_t, dst)
            for m in range(4 * g, 4 * g + 4):
                yield lambda m=m: v_quantum(m)

        def proj_group(g):
            for q in proj_quanta(g):
                q()

        def attn_group(g, only_pair=None, step=lambda: None):
            """Attention for query block g (512 cols) over head pairs."""
            gq = g * 512
            nt = 4 * g + 4
            flush_pend()
            pairs = range(4) if only_pair is None else [only_pair]
            for p in pairs:
                av = av_ps.tile([DK + 1, 1024], f32, tag="av", name="av")
                for t in range(nt):
                    step()
                    v = t - 4 * g
                    c0 = 128 * v if v > 0 else 0
                    w = 512 - c0
                    sc = sc_ps.tile([P, 1024], f32, tag="sc", name="sc")
                    for hh in range(2):
                        nc.tensor.matmul(
                            sc[:, 512 * hh + c0:512 * hh + 512],
                            kt_sb[p][64 * hh:64 * hh + 64,
                                     t * P:(t + 1) * P],
                            qt_sb[p][64 * hh:64 * hh + 64,
                                     gq + c0:gq + 512],
                            start=True, stop=True)
                    pt = ptp.tile([P, 1024], bf16, tag="pt", name="pt")
                    sc_v = bass.AP(tensor=sc.tensor, offset=sc.offset + c0,
                                   ap=[sc.ap[0], [512, 2], [1, w]])
                    pt_v = bass.AP(tensor=pt.tensor, offset=pt.offset + c0,
                                   ap=[pt.ap[0], [512, 2], [1, w]])
                    nc.scalar.activation(pt_v, sc_v, Exp)
                    if v >= 0:
                        pm = bass.AP(tensor=pt.tensor, offset=pt.offset + c0,
                                     ap=[pt.ap[0], [512, 2], [1, P]])
                        mk = bass.AP(tensor=mask_sb.tensor,
                                     offset=mask_sb.offset,
                                     ap=[mask_sb.ap[0], [0, 2], [1, P]])
                        nc.vector.tensor_mul(pm, pm, mk)
                    for hh in range(2):
                        nc.tensor.matmul(
                            av[:, 512 * hh + c0:512 * hh + 512],
                            v_sb[t][:, (2 * p + hh) * (DK + 1):
                                    (2 * p + hh) * (DK + 1) + DK + 1],
                            pt[:, 512 * hh + c0:512 * hh + 512],
                            start=(t == 0), stop=(t == nt - 1))
                # evacuate av to SBUF promptly so the next pair's first AV
                # matmul doesn't wait on the whole norm chain reading PSUM
                avs = normp.tile([DK + 1, 1024], f32, tag="avs", name="avs")
                nc.vector.tensor_copy(avs[:, 0:512], av[:, 0:512])
                nc.vector.tensor_copy(avs[:, 512:1024], av[:, 512:1024])
                rcp = normp.tile([1, 1024], f32, tag="rcp", name="rcp")
                nc.vector.reciprocal(rcp, avs[DK:DK + 1, :])
                rmat = normp.tile([DK, 1024], f32, tag="rmat", name="rmat")
                nc.gpsimd.partition_broadcast(rmat, rcp, channels=DK)
                nc.vector.tensor_mul(ot_sb[p][0:64, gq:gq + 512],
                                     avs[0:DK, 0:512], rmat[:, 0:512])
                # engines can't write other partitions; DMA moves the odd
                # head's half down to rows 64-127
                nrm2 = normp.tile([DK, 512], bf16, tag="nrm2", name="nrm2")
                nc.vector.tensor_mul(nrm2, avs[0:DK, 512:1024],
                                     rmat[:, 512:1024])
                q = nc.sync if p % 2 == 0 else nc.scalar
                q.dma_start(out=ot_sb[p][64:128, gq:gq + 512], in_=nrm2)

        og_live = {}

        def oproj_quantum(m, nb):
            flush_pend()
            ps = mm_ps.tile([P, 512], f32, tag="mm", name="mm")
            for k in range(4):
                nc.tensor.matmul(
                    ps, ot_sb[k][:, m * P:(m + 1) * P],
                    wo_all[:, k * D_MODEL + nb * 512:
                           k * D_MODEL + (nb + 1) * 512],
                    start=(k == 0), stop=(k == 3))
            if nb == 0:
                og_live[m] = ostg.tile([P, D_MODEL], bf16, tag="og",
                                       name="og")
            og = og_live[m]
            nc.vector.tensor_copy(og[:, nb * 512:(nb + 1) * 512], ps)
            if nb == 1:
                q = nc.sync if m % 2 == 0 else nc.scalar
                q.dma_start(out=out[m * P:(m + 1) * P, :],
                            in_=og_live.pop(m))

        def oproj_quanta(g):
            for m in range(4 * g, 4 * g + 4):
                for nb in range(2):
                    yield lambda m=m, nb=nb: oproj_quantum(m, nb)

        # Fine-grained software pipelining: engines run their streams in
        # strict emission order, so proj(2,3)/O-proj matmuls are emitted as
        # filler quanta INTO the attn kv-loops.
        for _rep in range(int(os.environ.get("BODY_REPEAT", "1"))):
            if _rep:
                for c in range(4):
                    x_chunk_dma(c)

            fill = []
            state = {"n": 0, "cad": 3}

            def step():
                state["n"] += 1
                if state["n"] % state["cad"] == 0 and fill:
                    fill.pop(0)()

            proj_group(0)
            proj_group(1)
            fill.extend(proj_quanta(2))
            attn_group(0, step=step)
            fill.extend(proj_quanta(3))
            attn_group(1, step=step)
            fill.extend(oproj_quanta(0))
            attn_group(2, step=step)
            fill.extend(oproj_quanta(1))
            state["cad"] = 2
            for p in range(4):
                attn_group(3, only_pair=p, step=step)
                if p >= 2:
                    fill.extend(oproj_quanta(p))
            while fill:
                fill.pop(0)()

    nc.compile()
    return nc


def _get_nc():
    if "nc" not in _CACHE:
        _CACHE["nc"] = _build()
    return _CACHE["nc"]


def _prep_core_inputs(q_proj_weight, k_proj_weight, v_proj_weight,
                      o_proj_weight, in_features, token_positions):
    """Host-side sharding: returns the list of 8 per-core input dicts."""
    import ml_dtypes
    bf = ml_dtypes.bfloat16

    x = np.asarray(in_features, np.float32)
    wqf = np.asarray(q_proj_weight, np.float32)
    wkf = np.asarray(k_proj_weight, np.float32)
    wvf = np.asarray(v_proj_weight, np.float32)
    wof = np.asarray(o_proj_weight, np.float32)
    tp = np.asarray(token_positions).astype(np.float64)

    # de-interleave permutation within each head: [e0..e31, o0..o31]
    perm = np.concatenate(
        [64 * h + np.concatenate([2 * np.arange(32), 2 * np.arange(32) + 1])
         for h in range(HPC)])

    inv = 1.0 / (THETA ** (np.arange(HALF, dtype=np.float64) / HALF))
    fr = tp[:, None] * inv[None, :]                      # [S, 32]
    cosT = np.cos(fr).T                                  # [32, S]
    sinT = np.sin(fr).T
    ctab = np.tile(cosT, (4, 1)).astype(bf)              # [128, S]
    stab = np.concatenate([sinT, -sinT, sinT, -sinT], axis=0).astype(bf)

    kv = np.arange(P)[:, None]
    j = np.arange(P)[None, :]
    maskt = (j >= kv).astype(bf)                         # [128, 128]
    swapm = (kv == (j ^ 32)).astype(bf)                  # 32-row pair swap

    in_maps = []
    for c in range(N_CORES):
        b, hg = c // 2, c % 2
        rows = slice(hg * DOUT, (hg + 1) * DOUT)
        in_maps.append({
            "xT": np.ascontiguousarray(x[b].T).astype(bf),
            "wq": np.ascontiguousarray((wqf[rows][perm] * SCALE).T).astype(bf),
            "wk": np.ascontiguousarray(wkf[rows][perm].T).astype(bf),
            "wv": np.ascontiguousarray(wvf[rows].T).astype(bf),
            "wo": np.ascontiguousarray(wof[:, rows].T).astype(bf),
            "ctab": ctab,
            "stab": stab,
            "maskt": maskt,
            "swapm": swapm,
        })
    return in_maps


def kernel(q_proj_weight, k_proj_weight, v_proj_weight, o_proj_weight,
           in_features, token_positions):
    from concourse.bass_utils import run_bass_kernel_spmd

    nc = _get_nc()
    in_maps = _prep_core_inputs(q_proj_weight, k_proj_weight, v_proj_weight,
                                o_proj_weight, in_features, token_positions)
    trace = bool(int(os.environ.get("KBENCH_TRACE", "0")))
    res = run_bass_kernel_spmd(nc, in_maps, list(range(N_CORES)), trace=trace)
    _CACHE["last_results"] = res
    if res.exec_time_ns is not None:
        _CACHE["exec_time_ns"] = res.exec_time_ns

    outp = np.empty((B, S, D_MODEL), np.float32)
    for b in range(B):
        outp[b] = (res.results[2 * b]["out"].astype(np.float32)
                   + res.results[2 * b + 1]["out"].astype(np.float32))
    return outp
